# revision 11
# baseline (speedup 1.0000x reference)
"""Trainium2 Bass kernel for nn_AttentionLayer (self-attention over 64x64 images).

Computation (per batch image b):
    xf = x[b].reshape(C, N)                     # C=256, N=4096
    q = BN(Wq @ xf + bq)   -> [32, N]
    k = BN(Wk @ xf + bk)   -> [32, N]
    v = BN(Wv @ xf + bv)   -> [256, N]
    attn = softmax_j(q^T k) -> [N, N]
    out = v @ attn^T        -> [256, N]
    y = gamma * out + xf

Sharding: 8 cores = 4 batches x 2 query-row halves; no collectives.

Host-side algebra folds (all exact):
  - BN folded into weights/bias.
  - k bias drops entirely (adds a per-query constant to every logit row ->
    cancels in softmax); q bias kept (its logit term varies over keys).
  - v bias passes through softmax unchanged (rows sum to 1), so
    y = (gamma*vhat @ P)*recip + (x + gamma*bv); gamma folds into Wv and the
    residual bias rb = gamma*bv is added per-channel in the epilogue.
  - Each core's xbh has its own query-half columns FIRST (keys are
    order-invariant), so the query slice is a view of the key tensor and the
    bf16 x tile doubles as the residual.

Device (per core, all matmuls bf16 in / f32 psum):
  - Input DMAs in 3 column waves issued from three different queues
    (sync/gpsimd/scalar) so the ~0.7us per-issue cost is paid in parallel
    and the first projection matmul starts ~9.5us in.
  - q/k proj: 2 col-group replica matmuls -> [64, N*] (col-group pairs
    stream near-concurrently on the PE); q evicted via ACT Identity+bias,
    k via DVE copy.  Bands 64-127 are then filled by one SBUF->SBUF DMA
    per tensor, so S rounds can use 4 distinct PE row bands.
  - V^T[j, c] = x^T Wv^T computed directly in [j, c] layout.
  - Main loop over 4 i-blocks x 16 rounds of 2 key-chunks: S^T row-band
    matmul pair -> exp on ScalarE (no max subtraction; |S|<60 fits f32/bf16)
    -> P^T bf16 -> out psum accumulation + bf16 rowsum chains on DVE.
    Consecutive rounds use different PE row-band pairs (0/32 vs 64/96), so
    the adjacently emitted S pairs of rounds sr+2 and sr+3 stream 4-way
    concurrently.
  - Epilogue per i-block is split so no PE instruction ever waits on the
    rowsum->reciprocal chain: at the block's last round only the psum
    eviction happens; the rowsum matmuls + reciprocal + GpSimd
    partition_broadcast flush after the NEXT block's round-0 outs, and the
    final (out*recip + rb) + x + DMA after round-1 outs.
"""

import numpy as np
from contextlib import ExitStack

import ml_dtypes
import concourse.bass as bass
import concourse.bass_isa as bass_isa
import concourse.mybir as mybir
import concourse.tile as tile
from concourse import bacc
from concourse.bass_utils import run_bass_kernel_spmd

B, C, H, W = 4, 256, 64, 64
N = H * W            # 4096 tokens per image
CQ = C // 8          # 32 q/k channels
NQ = N // 2          # 2048 query tokens per core
EPS = 1e-5
P = 128
IB = 512             # i-block (psum bank of f32)
NJC = N // P         # 32 j-chunks
NSR = NJC // 2       # 16 rounds per i-block (2 j-chunks each)
NCORES = 8
NWARM = 30

f32 = mybir.dt.float32
bf16 = mybir.dt.bfloat16
FT = mybir.ActivationFunctionType
ALU = mybir.AluOpType
BF = ml_dtypes.bfloat16

_CACHE = {}


def _build():
    nc = bacc.Bacc("TRN2", target_bir_lowering=False, debug=False,
                   num_devices=NCORES)
    xbh = nc.dram_tensor("xbh", [C, N], bf16, kind="ExternalInput").ap()
    wqT = nc.dram_tensor("wqT", [C, CQ], bf16, kind="ExternalInput").ap()
    wkT = nc.dram_tensor("wkT", [C, CQ], bf16, kind="ExternalInput").ap()
    wvT = nc.dram_tensor("wvT", [C, C], bf16, kind="ExternalInput").ap()
    bq2 = nc.dram_tensor("bq2", [2 * CQ, 1], f32, kind="ExternalInput").ap()
    rbh = nc.dram_tensor("rbh", [P, 2], f32, kind="ExternalInput").ap()
    y = nc.dram_tensor("y", [C, NQ], bf16, kind="ExternalOutput").ap()

    with tile.TileContext(nc) as tc, ExitStack() as ctx:
        const = ctx.enter_context(tc.tile_pool(name="const", bufs=1))
        ones_col = const.tile([P, 1], bf16)
        nc.vector.memset(ones_col[:], 1.0)
        ones_row = const.tile([1, P], bf16)
        nc.vector.memset(ones_row[:], 1.0)

        wq_sb = const.tile([P, 2, CQ], bf16)
        wk_sb = const.tile([P, 2, CQ], bf16)
        wv_sb = const.tile([P, 2, C], bf16)
        bq_sb = const.tile([2 * CQ, 1], f32)
        rb_sb = const.tile([P, 2], f32)

        garb = const.tile([P, 64], bf16)
        nc.vector.memset(garb[:], 0.5)

        xpool = ctx.enter_context(tc.tile_pool(name="x", bufs=1))
        xb_sb = [xpool.tile([P, N], bf16, name=f"xbsb{cc}") for cc in range(2)]

        qkv = ctx.enter_context(tc.tile_pool(name="qkv", bufs=1))
        qrep = qkv.tile([P, NQ], bf16)   # 4 row-band replicas of q
        krep = qkv.tile([P, N], bf16)    # 4 row-band replicas of k
        vt_sb = qkv.tile([P, NJC, C], bf16)  # V^T as [j-in-chunk, jc, c]

        # ---- input DMA: 4 column waves, xb0 on sync / xb1 on gpsimd /
        # weights on scalar.  Each wave overlaps the next by ONE column:
        # the WAW forces wave w+1 to start only after wave w completes, so
        # the first wave drains at full DMA bandwidth instead of sharing
        # it with the whole input, and the q-proj can start ~2us earlier.
        XW = (0, 512, 1536, 2816, N)
        nc.scalar.dma_start(out=wq_sb[:],
                            in_=wqT.rearrange("(k p) m -> p k m", k=2))
        for w in range(4):
            hi = min(XW[w + 1] + 1, N)
            nc.sync.dma_start(out=xb_sb[0][:, XW[w]:hi],
                              in_=xbh[0:P, XW[w]:hi])
            nc.gpsimd.dma_start(out=xb_sb[1][:, XW[w]:hi],
                                in_=xbh[P:C, XW[w]:hi])
        nc.scalar.dma_start(out=bq_sb[:], in_=bq2[:])
        nc.scalar.dma_start(out=wk_sb[:],
                            in_=wkT.rearrange("(k p) m -> p k m", k=2))
        nc.scalar.dma_start(out=wv_sb[:],
                            in_=wvT.rearrange("(k p) m -> p k m", k=2))
        nc.scalar.dma_start(out=rb_sb[:], in_=rbh[:])

        # PE warmup during the input-DMA window: dependency-free tiny
        # matmuls keep the PE busy past the HAM activity window so the
        # projection phase starts at the full clock.
        with tc.tile_pool(name="warm_ps", bufs=1, space="PSUM") as wps:
            wtile = wps.tile([1, 64], f32, name="warm")
            for _ in range(NWARM):
                nc.tensor.matmul(wtile[:], lhsT=ones_col[:], rhs=garb[:],
                                 start=True, stop=True)

        # ---- projections, emitted in column-availability order so the
        # PE is never waiting on a later DMA wave while earlier-column
        # work exists ----
        def emit_q(nb, pps):
            ps = pps.tile([2 * CQ, IB], f32, name="qps", tag="pps")
            for g in range(2):
                for k in range(2):
                    nc.tensor.matmul(
                        ps[g * CQ:(g + 1) * CQ, :],
                        lhsT=wq_sb[:, k, :],
                        rhs=xb_sb[k][:, nb * IB:(nb + 1) * IB],
                        start=(k == 0), stop=(k == 1),
                        tile_position=(0, g * CQ))
            nc.scalar.activation(
                out=qrep[0:2 * CQ, nb * IB:(nb + 1) * IB], in_=ps[:],
                func=FT.Identity, bias=bq_sb[:])

        def emit_k(nb, pps):
            ps = pps.tile([2 * CQ, IB], f32, name="kps", tag="pps")
            for g in range(2):
                for k in range(2):
                    nc.tensor.matmul(
                        ps[g * CQ:(g + 1) * CQ, :],
                        lhsT=wk_sb[:, k, :],
                        rhs=xb_sb[k][:, nb * IB:(nb + 1) * IB],
                        start=(k == 0), stop=(k == 1),
                        tile_position=(0, g * CQ))
            nc.vector.tensor_copy(
                out=krep[0:2 * CQ, nb * IB:(nb + 1) * IB], in_=ps[:])

        def emit_v(jc, pps):
            # V^T = x^T Wv^T in [j, c] layout (gamma folded into Wv on host)
            ps = pps.tile([P, C], f32, name="vps", tag="pps")
            for k in range(2):
                nc.tensor.matmul(
                    ps[:],
                    lhsT=xb_sb[k][:, jc * P:(jc + 1) * P],
                    rhs=wv_sb[:, k, :],
                    start=(k == 0), stop=(k == 1))
            nc.vector.tensor_copy(out=vt_sb[:, jc, :], in_=ps[:])

        with tc.tile_pool(name="proj_ps", bufs=4, space="PSUM") as pps:
            for step in ("q0", "v0", "v1", "v2", "v3",
                         "q1", "q2", "v4", "v5", "v6", "v7", "v8", "v9",
                         "v10", "v11", "q3", "QR",
                         "k0", "k1", "k2", "v12", "v13", "v14", "v15",
                         "k3", "KR1", "k4", "v16", "v17", "v18", "v19", "v20",
                         "v21", "k5", "k6", "k7", "KR2",
                         "v22", "v23", "v24", "v25", "v26", "v27", "v28",
                         "v29", "v30", "v31"):
                if step == "QR":    # replicate q bands 0/1 -> 2/3
                    nc.gpsimd.dma_start(out=qrep[2 * CQ:P, :],
                                        in_=qrep[0:2 * CQ, :])
                elif step == "KR1":  # replicate k bands 0/1 -> 2/3 (1st half)
                    nc.sync.dma_start(out=krep[2 * CQ:P, 0:2048],
                                      in_=krep[0:2 * CQ, 0:2048])
                elif step == "KR2":  # replicate k bands 0/1 -> 2/3 (2nd half)
                    nc.sync.dma_start(out=krep[2 * CQ:P, 2048:N],
                                      in_=krep[0:2 * CQ, 2048:N])
                elif step[0] == "q":
                    emit_q(int(step[1:]), pps)
                elif step[0] == "k":
                    emit_k(int(step[1:]), pps)
                else:
                    emit_v(int(step[1:]), pps)

        # ---- attention main loop ----
        sp_ps = ctx.enter_context(tc.tile_pool(name="sp_ps", bufs=3, space="PSUM"))
        out_ps = ctx.enter_context(tc.tile_pool(name="out_ps", bufs=1, space="PSUM"))
        pp_pool = ctx.enter_context(tc.tile_pool(name="ppp", bufs=4))
        acc_pool = ctx.enter_context(tc.tile_pool(name="accp", bufs=6))
        osb_pool = ctx.enter_context(tc.tile_pool(name="osbp", bufs=2))
        ysb_pool = ctx.enter_context(tc.tile_pool(name="ysbp", bufs=4))
        rec_pool = ctx.enter_context(tc.tile_pool(name="recp", bufs=2))
        bc_pool = ctx.enter_context(tc.tile_pool(name="bcp", bufs=2))

        NIB = NQ // IB
        NSRT = NIB * NSR   # total rounds

        def emit_s(sr):
            """S^T matmul pair for round sr: two concurrent 32-row-band MMs.

            Even rounds use PE row bands 0/32, odd rounds 64/96, so the two
            adjacently emitted pairs stream 4-way concurrently.
            """
            ib, lsr = divmod(sr, NSR)
            i0 = ib * IB
            base = (sr % 2) * 2 * CQ
            sp = sp_ps.tile([P, 2, IB], f32, name="sp")
            for g in range(2):
                jc = 2 * lsr + g
                bb = base + g * CQ
                nc.tensor.matmul(
                    sp[:, g, :],
                    lhsT=krep[bb:bb + CQ, jc * P:(jc + 1) * P],
                    rhs=qrep[bb:bb + CQ, i0:i0 + IB],
                    start=True, stop=True,
                    tile_position=(bb, 0))
            return sp

        def flush1(st):
            """Rowsum reduction + reciprocal, already broadcast.

            Non-last blocks run entirely on GpSimd/DVE SBUF tiles (no psum,
            no PE): merge chains, partition_all_reduce (every partition gets
            the sum), add the two chunk groups, reciprocal.  This keeps the
            sp psum slots free, so the S-pair rotation never stalls at
            i-block boundaries.  The last block uses the PE path (rowsum
            matmuls + rank-1 broadcast) for minimum tail latency."""
            if st["last"]:
                epi, accB = st["epi"], st["accB"]
                rs = epi[0:1, 0, :]
                for g in range(2):
                    nc.tensor.matmul(rs, lhsT=ones_col[:],
                                     rhs=accB[:, g, :],
                                     start=False, stop=(g == 1))
                recip = rec_pool.tile([1, IB], f32, name="recip")
                nc.vector.reciprocal_approx_fast(out=recip[:], in_=rs)
                recb = rec_pool.tile([1, IB], bf16, name="recb")
                nc.vector.tensor_copy(out=recb[:], in_=recip[:])
                bc = epi[:, 1, :]
                nc.tensor.matmul(bc, lhsT=ones_row[:], rhs=recb[:],
                                 start=True, stop=True)
                bcs = bc_pool.tile([P, IB], bf16, name="bcs")
                nc.vector.tensor_copy(out=bcs[:], in_=bc)
            else:
                accA, accB = st["accA"], st["accB"]
                accM = acc_pool.tile([P, 2, IB], bf16, name="accM")
                nc.gpsimd.tensor_add(accM[:], accA[:], accB[:])
                parM = bc_pool.tile([P, 2, IB], f32, name="parM", tag="parM")
                nc.gpsimd.partition_all_reduce(
                    out_ap=parM[:], in_ap=accM[:], channels=P,
                    reduce_op=bass_isa.ReduceOp.add)
                parG = rec_pool.tile([P, IB], f32, name="parG", tag="parG")
                nc.gpsimd.tensor_add(parG[:], parM[:, 0, :], parM[:, 1, :])
                rcf = rec_pool.tile([P, IB], f32, name="rcf", tag="rcf")
                nc.vector.reciprocal_approx_fast(out=rcf[:], in_=parG[:])
                bcs = bc_pool.tile([P, IB], bf16, name="bcs")
                nc.vector.tensor_copy(out=bcs[:], in_=rcf[:])
            st["bcs"] = bcs

        def flush2(st):
            """Normalize + residual + output DMA, all bf16 so DVE runs in
            2x mode and the boundary rounds don't stall the exp stream."""
            bcs, osb, i0 = st["bcs"], st["osb"], st["i0"]
            for cc in range(2):
                tmp = ysb_pool.tile([P, IB], bf16, name="tmp")
                nc.vector.tensor_mul(tmp[:], osb[:, cc, :], bcs[:])
                ysb = ysb_pool.tile([P, IB], bf16, name="ysb")
                nc.vector.scalar_tensor_tensor(
                    out=ysb[:], in0=tmp[:], scalar=rb_sb[:, cc:cc + 1],
                    in1=xb_sb[cc][:, i0:i0 + IB],
                    op0=ALU.add, op1=ALU.add)
                nc.sync.dma_start(out=y[cc * P:(cc + 1) * P, i0:i0 + IB],
                                  in_=ysb[:])

        # software pipeline: S for round sr+2 is emitted right after round
        # sr's out matmuls, so its LDWEIGHTS prefetch during them and the
        # PE never waits on the exp stream.
        sps = {0: emit_s(0), 1: emit_s(1)}
        outp = None
        accs = None
        pending = []
        for sr in range(NSRT):
            ib, lsr = divmod(sr, NSR)
            i0 = ib * IB
            if lsr == 0:
                outp = out_ps.tile([P, 2, IB], f32, name="outp")
                accs = [None, None]
            ch = lsr // (NSR // 2)   # rowsum chain A: rounds 0-7, B: 8-15
            if lsr % (NSR // 2) == 0:
                # chain start: exp writes the accumulator tile directly
                ppt = acc_pool.tile([P, 2, IB], bf16, name=f"acc{ch}")
                accs[ch] = ppt
                nc.scalar.activation(out=ppt[:], in_=sps.pop(sr)[:],
                                     func=FT.Exp)
            else:
                ppt = pp_pool.tile([P, 2, IB], bf16, name="pp")
                nc.scalar.activation(out=ppt[:], in_=sps.pop(sr)[:],
                                     func=FT.Exp)
                nc.vector.tensor_add(accs[ch][:], accs[ch][:], ppt[:])
            # S pairs for rounds sr+2 and sr+3 are emitted together (one
            # tile-mode run, LDWs pipeline) before this round's out matmuls;
            # with sp bufs=3 their psum WAR is already satisfied, so the exp
            # stream never waits on S production.
            if lsr % 2 == 0:
                for dd in (2, 3):
                    if sr + dd < NSRT:
                        sps[sr + dd] = emit_s(sr + dd)
            for g in range(2):
                jc = 2 * lsr + g
                for cc in range(2):
                    nc.tensor.matmul(
                        outp[:, cc, :],
                        lhsT=vt_sb[:, jc, cc * P:(cc + 1) * P],
                        rhs=ppt[:, g, :],
                        start=(lsr == 0 and g == 0),
                        stop=(lsr == NSR - 1 and g == 1))
            # previous block's deferred epilogue: the reduction chain after
            # this block's round-0 outs, the normalize+store after round-4
            # outs (the GpSimd reduction takes ~5us to produce bcs).
            if pending and ((lsr == 0 and len(pending) == 2) or lsr == 4):
                st, fn = pending.pop(0)
                fn(st)
            # last block only: PE rowsum chain A at lsr 13 (the psum epi
            # tile would otherwise pinch the sp slot rotation at block
            # boundaries; non-last blocks reduce on GpSimd instead).
            if lsr == NSR - 3 and ib == NIB - 1:
                epi = sp_ps.tile([P, 2, IB], f32, name="sp")
                rs = epi[0:1, 0, :]
                for g in range(2):
                    nc.tensor.matmul(rs, lhsT=ones_col[:],
                                     rhs=accs[0][:, g, :],
                                     start=(g == 0), stop=False)
                accs.append(epi)
            if lsr < NSR - 1:
                continue
            # ---- end of i-block: evict psum, defer the rest ----
            # evict psum so next i-block's matmuls proceed (one fused DVE
            # copy: cheaper than two, and ACT must stay on the exp stream)
            osb = osb_pool.tile([P, 2, IB], bf16, name="osb")
            nc.vector.tensor_copy(out=osb[:], in_=outp[:])
            st = {"epi": accs[2] if len(accs) > 2 else None,
                  "accA": accs[0], "accB": accs[1], "osb": osb, "i0": i0,
                  "last": sr == NSRT - 1}
            pending.append((st, flush1))
            pending.append((st, flush2))
        for st, fn in pending:   # last i-block epilogue
            fn(st)

    nc.compile()
    return nc


def _get_nc():
    if "nc" not in _CACHE:
        _CACHE["nc"] = _build()
    return _CACHE["nc"]


def _fold_bn(w, b, g, beta, mean, var):
    s = g / np.sqrt(var + EPS)
    return w * s[:, None], b * s + beta - mean * s


def _in_maps(inputs):
    gx = np.asarray(inputs["x"], np.float32)
    gamma = float(np.asarray(inputs["gamma"]).reshape(-1)[0])
    wq, bq_ = _fold_bn(*[np.asarray(inputs[k], np.float32) for k in
                         ("q_w", "q_b", "q_g", "q_beta", "q_mean", "q_var")])
    wk, _bk = _fold_bn(*[np.asarray(inputs[k], np.float32) for k in
                         ("k_w", "k_b", "k_g", "k_beta", "k_mean", "k_var")])
    wv, bv_ = _fold_bn(*[np.asarray(inputs[k], np.float32) for k in
                         ("v_w", "v_b", "v_g", "v_beta", "v_mean", "v_var")])
    wqT = np.ascontiguousarray(wq.T.astype(BF))
    wkT = np.ascontiguousarray(wk.T.astype(BF))
    wvT = np.ascontiguousarray((gamma * wv).T.astype(BF))
    bq2 = np.ascontiguousarray(np.tile(bq_.reshape(CQ, 1), (2, 1)))
    rbh = np.ascontiguousarray((gamma * bv_).reshape(2, P).T)
    maps = []
    for core in range(NCORES):
        b, h = divmod(core, 2)
        xf = gx[b].reshape(C, N).astype(BF)
        if h == 1:  # own query-half columns first; key order is irrelevant
            xf = np.concatenate([xf[:, NQ:], xf[:, :NQ]], axis=1)
        maps.append({
            "xbh": np.ascontiguousarray(xf),
            "wqT": wqT, "wkT": wkT, "wvT": wvT,
            "bq2": bq2, "rbh": rbh,
        })
    return maps


def _gather(results):
    out = np.empty((B, C, N), np.float32)
    for core in range(NCORES):
        b, h = divmod(core, 2)
        out[b][:, h * NQ:(h + 1) * NQ] = np.asarray(
            results[core]["y"]).astype(np.float32)
    return out.reshape(B, C, H, W)


def _run(inputs, **kw):
    nc = _get_nc()
    res = run_bass_kernel_spmd(nc, _in_maps(inputs),
                               core_ids=list(range(NCORES)), **kw)
    return res


def kernel(**inputs) -> np.ndarray:
    return _gather(_run(inputs).results)


# revision 12
# speedup vs baseline: 1.5456x; 1.5456x over previous
"""Trainium2 Bass kernel for nn_AttentionLayer (self-attention over 64x64 images).

Computation (per batch image b):
    xf = x[b].reshape(C, N)                     # C=256, N=4096
    q = BN(Wq @ xf + bq)   -> [32, N]
    k = BN(Wk @ xf + bk)   -> [32, N]
    v = BN(Wv @ xf + bv)   -> [256, N]
    attn = softmax_j(q^T k) -> [N, N]
    out = v @ attn^T        -> [256, N]
    y = gamma * out + xf

Sharding: 8 cores = 4 batches x 2 query-row halves; no collectives.

Host-side algebra folds (all exact):
  - BN folded into weights/bias.
  - k bias drops entirely (adds a per-query constant to every logit row ->
    cancels in softmax); q bias kept (its logit term varies over keys).
  - v bias passes through softmax unchanged (rows sum to 1), so
    y = (gamma*vhat @ P)*recip + (x + gamma*bv); gamma folds into Wv and the
    residual bias rb = gamma*bv is added per-channel in the epilogue.
  - Each core's xbh has its own query-half columns FIRST (keys are
    order-invariant), so the query slice is a view of the key tensor and the
    bf16 x tile doubles as the residual.

Device (per core, all matmuls bf16 in / f32 psum):
  - Input DMAs in 3 column waves issued from three different queues
    (sync/gpsimd/scalar) so the ~0.7us per-issue cost is paid in parallel
    and the first projection matmul starts ~9.5us in.
  - q/k proj: 2 col-group replica matmuls -> [64, N*] (col-group pairs
    stream near-concurrently on the PE); q evicted via ACT Identity+bias,
    k via DVE copy.  Bands 64-127 are then filled by one SBUF->SBUF DMA
    per tensor, so S rounds can use 4 distinct PE row bands.
  - V^T[j, c] = x^T Wv^T computed directly in [j, c] layout.
  - Main loop over 4 i-blocks x 16 rounds of 2 key-chunks: S^T row-band
    matmul pair -> exp on ScalarE (no max subtraction; |S|<60 fits f32/bf16)
    -> P^T bf16 -> out psum accumulation + bf16 rowsum chains on DVE.
    Consecutive rounds use different PE row-band pairs (0/32 vs 64/96), so
    the adjacently emitted S pairs of rounds sr+2 and sr+3 stream 4-way
    concurrently.
  - Epilogue per i-block is split so no PE instruction ever waits on the
    rowsum->reciprocal chain: at the block's last round only the psum
    eviction happens; the rowsum matmuls + reciprocal + GpSimd
    partition_broadcast flush after the NEXT block's round-0 outs, and the
    final (out*recip + rb) + x + DMA after round-1 outs.
"""

import numpy as np
from contextlib import ExitStack

import ml_dtypes
import concourse.bass as bass
import concourse.bass_isa as bass_isa
import concourse.mybir as mybir
import concourse.tile as tile
from concourse import bacc
from concourse.bass_utils import run_bass_kernel_spmd

B, C, H, W = 4, 256, 64, 64
N = H * W            # 4096 tokens per image
CQ = C // 8          # 32 q/k channels
NQ = N // 2          # 2048 query tokens per core
EPS = 1e-5
P = 128
IB = 512             # i-block (psum bank of f32)
NJC = N // P         # 32 j-chunks
NSR = NJC // 2       # 16 rounds per i-block (2 j-chunks each)
NCORES = 8
NWARM = 30

f32 = mybir.dt.float32
bf16 = mybir.dt.bfloat16
FT = mybir.ActivationFunctionType
ALU = mybir.AluOpType
BF = ml_dtypes.bfloat16

_CACHE = {}


def _build():
    nc = bacc.Bacc("TRN2", target_bir_lowering=False, debug=False,
                   num_devices=NCORES)
    xbh = nc.dram_tensor("xbh", [C, N], bf16, kind="ExternalInput").ap()
    wqT = nc.dram_tensor("wqT", [C, CQ], bf16, kind="ExternalInput").ap()
    wkT = nc.dram_tensor("wkT", [C, CQ], bf16, kind="ExternalInput").ap()
    wvT = nc.dram_tensor("wvT", [C, C], bf16, kind="ExternalInput").ap()
    bq2 = nc.dram_tensor("bq2", [2 * CQ, 1], f32, kind="ExternalInput").ap()
    rbh = nc.dram_tensor("rbh", [P, 2], f32, kind="ExternalInput").ap()
    y = nc.dram_tensor("y", [C, NQ], bf16, kind="ExternalOutput").ap()

    with tile.TileContext(nc) as tc, ExitStack() as ctx:
        const = ctx.enter_context(tc.tile_pool(name="const", bufs=1))
        ones_col = const.tile([P, 1], bf16)
        nc.vector.memset(ones_col[:], 1.0)
        ones_row = const.tile([1, P], bf16)
        nc.vector.memset(ones_row[:], 1.0)

        wq_sb = const.tile([P, 2, CQ], bf16)
        wk_sb = const.tile([P, 2, CQ], bf16)
        wv_sb = const.tile([P, 2, C], bf16)
        bq_sb = const.tile([2 * CQ, 1], f32)
        rb_sb = const.tile([P, 2], f32)

        garb = const.tile([P, 64], bf16)
        nc.vector.memset(garb[:], 0.5)

        xpool = ctx.enter_context(tc.tile_pool(name="x", bufs=1))
        xb_sb = [xpool.tile([P, N], bf16, name=f"xbsb{cc}") for cc in range(2)]

        qkv = ctx.enter_context(tc.tile_pool(name="qkv", bufs=1))
        qrep = qkv.tile([P, NQ], bf16)   # 4 row-band replicas of q
        krep = qkv.tile([P, N], bf16)    # 4 row-band replicas of k
        vt_sb = qkv.tile([P, NJC, C], bf16)  # V^T as [j-in-chunk, jc, c]

        # ---- input DMA: 4 column waves, xb0 on sync / xb1 on gpsimd /
        # weights on scalar.  Each wave overlaps the next by ONE column:
        # the WAW forces wave w+1 to start only after wave w completes, so
        # the first wave drains at full DMA bandwidth instead of sharing
        # it with the whole input, and the q-proj can start ~2us earlier.
        XW = (0, 512, 1536, 2816, N)
        nc.scalar.dma_start(out=wq_sb[:],
                            in_=wqT.rearrange("(k p) m -> p k m", k=2))
        for w in range(4):
            hi = min(XW[w + 1] + 1, N)
            nc.sync.dma_start(out=xb_sb[0][:, XW[w]:hi],
                              in_=xbh[0:P, XW[w]:hi])
            nc.gpsimd.dma_start(out=xb_sb[1][:, XW[w]:hi],
                                in_=xbh[P:C, XW[w]:hi])
        nc.scalar.dma_start(out=bq_sb[:], in_=bq2[:])
        nc.scalar.dma_start(out=wk_sb[:],
                            in_=wkT.rearrange("(k p) m -> p k m", k=2))
        nc.scalar.dma_start(out=wv_sb[:],
                            in_=wvT.rearrange("(k p) m -> p k m", k=2))
        nc.scalar.dma_start(out=rb_sb[:], in_=rbh[:])

        # PE warmup during the input-DMA window: dependency-free tiny
        # matmuls keep the PE busy past the HAM activity window so the
        # projection phase starts at the full clock.
        with tc.tile_pool(name="warm_ps", bufs=1, space="PSUM") as wps:
            wtile = wps.tile([1, 64], f32, name="warm")
            for _ in range(NWARM):
                nc.tensor.matmul(wtile[:], lhsT=ones_col[:], rhs=garb[:],
                                 start=True, stop=True)

        # ---- projections, emitted in column-availability order so the
        # PE is never waiting on a later DMA wave while earlier-column
        # work exists ----
        def emit_q(nb, pps):
            ps = pps.tile([2 * CQ, IB], f32, name="qps", tag="pps")
            for g in range(2):
                for k in range(2):
                    nc.tensor.matmul(
                        ps[g * CQ:(g + 1) * CQ, :],
                        lhsT=wq_sb[:, k, :],
                        rhs=xb_sb[k][:, nb * IB:(nb + 1) * IB],
                        start=(k == 0), stop=(k == 1),
                        tile_position=(0, g * CQ))
            nc.scalar.activation(
                out=qrep[0:2 * CQ, nb * IB:(nb + 1) * IB], in_=ps[:],
                func=FT.Identity, bias=bq_sb[:])

        def emit_k(nb, pps):
            ps = pps.tile([2 * CQ, IB], f32, name="kps", tag="pps")
            for g in range(2):
                for k in range(2):
                    nc.tensor.matmul(
                        ps[g * CQ:(g + 1) * CQ, :],
                        lhsT=wk_sb[:, k, :],
                        rhs=xb_sb[k][:, nb * IB:(nb + 1) * IB],
                        start=(k == 0), stop=(k == 1),
                        tile_position=(0, g * CQ))
            nc.vector.tensor_copy(
                out=krep[0:2 * CQ, nb * IB:(nb + 1) * IB], in_=ps[:])

        def emit_v(jc, pps):
            # V^T = x^T Wv^T in [j, c] layout (gamma folded into Wv on host)
            ps = pps.tile([P, C], f32, name="vps", tag="pps")
            for k in range(2):
                nc.tensor.matmul(
                    ps[:],
                    lhsT=xb_sb[k][:, jc * P:(jc + 1) * P],
                    rhs=wv_sb[:, k, :],
                    start=(k == 0), stop=(k == 1))
            nc.vector.tensor_copy(out=vt_sb[:, jc, :], in_=ps[:])

        with tc.tile_pool(name="proj_ps", bufs=4, space="PSUM") as pps:
            for step in ("q0", "v0", "v1", "v2", "v3",
                         "q1", "q2", "v4", "v5", "v6", "v7", "v8", "v9",
                         "v10", "v11", "q3", "QR",
                         "k0", "k1", "k2", "v12", "v13", "v14", "v15",
                         "k3", "KR1", "k4", "v16", "v17", "v18", "v19", "v20",
                         "v21", "k5", "k6", "k7", "KR2",
                         "v22", "v23", "v24", "v25", "v26", "v27", "v28",
                         "v29", "v30", "v31"):
                if step == "QR":    # replicate q bands 0/1 -> 2/3
                    nc.gpsimd.dma_start(out=qrep[2 * CQ:P, :],
                                        in_=qrep[0:2 * CQ, :])
                elif step == "KR1":  # replicate k bands 0/1 -> 2/3 (1st half)
                    nc.sync.dma_start(out=krep[2 * CQ:P, 0:2048],
                                      in_=krep[0:2 * CQ, 0:2048])
                elif step == "KR2":  # replicate k bands 0/1 -> 2/3 (2nd half)
                    nc.sync.dma_start(out=krep[2 * CQ:P, 2048:N],
                                      in_=krep[0:2 * CQ, 2048:N])
                elif step[0] == "q":
                    emit_q(int(step[1:]), pps)
                elif step[0] == "k":
                    emit_k(int(step[1:]), pps)
                else:
                    emit_v(int(step[1:]), pps)

        # ---- attention main loop ----
        sp_ps = ctx.enter_context(tc.tile_pool(name="sp_ps", bufs=3, space="PSUM"))
        out_ps = ctx.enter_context(tc.tile_pool(name="out_ps", bufs=1, space="PSUM"))
        pp_pool = ctx.enter_context(tc.tile_pool(name="ppp", bufs=4))
        acc_pool = ctx.enter_context(tc.tile_pool(name="accp", bufs=4))
        osb_pool = ctx.enter_context(tc.tile_pool(name="osbp", bufs=2))
        ysb_pool = ctx.enter_context(tc.tile_pool(name="ysbp", bufs=4))
        rec_pool = ctx.enter_context(tc.tile_pool(name="recp", bufs=2))
        bc_pool = ctx.enter_context(tc.tile_pool(name="bcp", bufs=2))

        NIB = NQ // IB
        NSRT = NIB * NSR   # total rounds

        def emit_s(sr):
            """S^T matmul pair for round sr: two concurrent 32-row-band MMs.

            Even rounds use PE row bands 0/32, odd rounds 64/96, so the two
            adjacently emitted pairs stream 4-way concurrently.
            """
            ib, lsr = divmod(sr, NSR)
            i0 = ib * IB
            base = (sr % 2) * 2 * CQ
            sp = sp_ps.tile([P, 2, IB], f32, name="sp")
            for g in range(2):
                jc = 2 * lsr + g
                bb = base + g * CQ
                nc.tensor.matmul(
                    sp[:, g, :],
                    lhsT=krep[bb:bb + CQ, jc * P:(jc + 1) * P],
                    rhs=qrep[bb:bb + CQ, i0:i0 + IB],
                    start=True, stop=True,
                    tile_position=(bb, 0))
            return sp

        def flush1(st):
            """Both rowsum chains + reciprocal + broadcast.  The psum epi
            tile is allocated HERE (after the next block's round-0 outs), so
            it never pinches the sp slot rotation across the boundary."""
            accA, accB = st["accA"], st["accB"]
            epi = sp_ps.tile([P, 2, IB], f32, name="sp")
            rs = epi[0:1, 0, :]
            for g in range(2):
                nc.tensor.matmul(rs, lhsT=ones_col[:], rhs=accA[:, g, :],
                                 start=(g == 0), stop=False)
            for g in range(2):
                nc.tensor.matmul(rs, lhsT=ones_col[:], rhs=accB[:, g, :],
                                 start=False, stop=(g == 1))
            recip = rec_pool.tile([1, IB], f32, name="recip")
            nc.vector.reciprocal_approx_fast(out=recip[:], in_=rs)
            recb = rec_pool.tile([1, IB], bf16, name="recb")
            nc.vector.tensor_copy(out=recb[:], in_=recip[:])
            if st["last"]:
                bc = epi[:, 1, :]
                nc.tensor.matmul(bc, lhsT=ones_row[:], rhs=recb[:],
                                 start=True, stop=True)
                bcs = bc_pool.tile([P, IB], bf16, name="bcs")
                nc.vector.tensor_copy(out=bcs[:], in_=bc)
            else:
                bcs = bc_pool.tile([P, IB], bf16, name="bcs")
                nc.gpsimd.partition_broadcast(out_ap=bcs[:], in_ap=recb[:],
                                              channels=P)
            st["bcs"] = bcs

        def flush2(st):
            """Normalize + residual + output DMA, all bf16 so DVE runs in
            2x mode and the boundary rounds don't stall the exp stream."""
            bcs, osb, i0 = st["bcs"], st["osb"], st["i0"]
            for cc in range(2):
                tmp = ysb_pool.tile([P, IB], bf16, name="tmp")
                nc.vector.tensor_mul(tmp[:], osb[:, cc, :], bcs[:])
                ysb = ysb_pool.tile([P, IB], bf16, name="ysb")
                nc.vector.scalar_tensor_tensor(
                    out=ysb[:], in0=tmp[:], scalar=rb_sb[:, cc:cc + 1],
                    in1=xb_sb[cc][:, i0:i0 + IB],
                    op0=ALU.add, op1=ALU.add)
                nc.sync.dma_start(out=y[cc * P:(cc + 1) * P, i0:i0 + IB],
                                  in_=ysb[:])

        # software pipeline: S for round sr+2 is emitted right after round
        # sr's out matmuls, so its LDWEIGHTS prefetch during them and the
        # PE never waits on the exp stream.
        sps = {0: emit_s(0), 1: emit_s(1)}
        outp = None
        accs = None
        pending = []
        for sr in range(NSRT):
            ib, lsr = divmod(sr, NSR)
            i0 = ib * IB
            if lsr == 0:
                outp = out_ps.tile([P, 2, IB], f32, name="outp")
                accs = [None, None]
            ch = lsr // (NSR // 2)   # rowsum chain A: rounds 0-7, B: 8-15
            if lsr % (NSR // 2) == 0:
                # chain start: exp writes the accumulator tile directly
                ppt = acc_pool.tile([P, 2, IB], bf16, name=f"acc{ch}")
                accs[ch] = ppt
                nc.scalar.activation(out=ppt[:], in_=sps.pop(sr)[:],
                                     func=FT.Exp)
            else:
                ppt = pp_pool.tile([P, 2, IB], bf16, name="pp")
                nc.scalar.activation(out=ppt[:], in_=sps.pop(sr)[:],
                                     func=FT.Exp)
                nc.vector.tensor_add(accs[ch][:], accs[ch][:], ppt[:])
            # S pairs for rounds sr+2 and sr+3 are emitted together (one
            # tile-mode run, LDWs pipeline) before this round's out matmuls;
            # with sp bufs=3 their psum WAR is already satisfied, so the exp
            # stream never waits on S production.
            if lsr % 2 == 0:
                for dd in (2, 3):
                    if sr + dd < NSRT:
                        sps[sr + dd] = emit_s(sr + dd)
            for g in range(2):
                jc = 2 * lsr + g
                for cc in range(2):
                    nc.tensor.matmul(
                        outp[:, cc, :],
                        lhsT=vt_sb[:, jc, cc * P:(cc + 1) * P],
                        rhs=ppt[:, g, :],
                        start=(lsr == 0 and g == 0),
                        stop=(lsr == NSR - 1 and g == 1))
            # previous block's deferred epilogue: the reduction after this
            # block's round-0 outs, the normalize+store after round-2 outs
            # (the recip->broadcast chain takes ~2us to produce bcs).
            if pending and ((lsr == 0 and len(pending) == 2) or lsr == 2):
                st, fn = pending.pop(0)
                fn(st)
            if lsr < NSR - 1:
                continue
            # ---- end of i-block: evict psum, defer the rest ----
            # evict psum so next i-block's matmuls proceed (one fused DVE
            # copy: cheaper than two, and ACT must stay on the exp stream)
            osb = osb_pool.tile([P, 2, IB], bf16, name="osb")
            nc.vector.tensor_copy(out=osb[:], in_=outp[:])
            st = {"accA": accs[0], "accB": accs[1], "osb": osb, "i0": i0,
                  "last": sr == NSRT - 1}
            pending.append((st, flush1))
            pending.append((st, flush2))
        for st, fn in pending:   # last i-block epilogue
            fn(st)

    nc.compile()
    return nc


def _get_nc():
    if "nc" not in _CACHE:
        _CACHE["nc"] = _build()
    return _CACHE["nc"]


def _fold_bn(w, b, g, beta, mean, var):
    s = g / np.sqrt(var + EPS)
    return w * s[:, None], b * s + beta - mean * s


def _in_maps(inputs):
    gx = np.asarray(inputs["x"], np.float32)
    gamma = float(np.asarray(inputs["gamma"]).reshape(-1)[0])
    wq, bq_ = _fold_bn(*[np.asarray(inputs[k], np.float32) for k in
                         ("q_w", "q_b", "q_g", "q_beta", "q_mean", "q_var")])
    wk, _bk = _fold_bn(*[np.asarray(inputs[k], np.float32) for k in
                         ("k_w", "k_b", "k_g", "k_beta", "k_mean", "k_var")])
    wv, bv_ = _fold_bn(*[np.asarray(inputs[k], np.float32) for k in
                         ("v_w", "v_b", "v_g", "v_beta", "v_mean", "v_var")])
    wqT = np.ascontiguousarray(wq.T.astype(BF))
    wkT = np.ascontiguousarray(wk.T.astype(BF))
    wvT = np.ascontiguousarray((gamma * wv).T.astype(BF))
    bq2 = np.ascontiguousarray(np.tile(bq_.reshape(CQ, 1), (2, 1)))
    rbh = np.ascontiguousarray((gamma * bv_).reshape(2, P).T)
    maps = []
    for core in range(NCORES):
        b, h = divmod(core, 2)
        xf = gx[b].reshape(C, N).astype(BF)
        if h == 1:  # own query-half columns first; key order is irrelevant
            xf = np.concatenate([xf[:, NQ:], xf[:, :NQ]], axis=1)
        maps.append({
            "xbh": np.ascontiguousarray(xf),
            "wqT": wqT, "wkT": wkT, "wvT": wvT,
            "bq2": bq2, "rbh": rbh,
        })
    return maps


def _gather(results):
    out = np.empty((B, C, N), np.float32)
    for core in range(NCORES):
        b, h = divmod(core, 2)
        out[b][:, h * NQ:(h + 1) * NQ] = np.asarray(
            results[core]["y"]).astype(np.float32)
    return out.reshape(B, C, H, W)


def _run(inputs, **kw):
    nc = _get_nc()
    res = run_bass_kernel_spmd(nc, _in_maps(inputs),
                               core_ids=list(range(NCORES)), **kw)
    return res


def kernel(**inputs) -> np.ndarray:
    return _gather(_run(inputs).results)


# revision 14
# speedup vs baseline: 1.5692x; 1.0153x over previous
"""Trainium2 Bass kernel for nn_AttentionLayer (self-attention over 64x64 images).

Computation (per batch image b):
    xf = x[b].reshape(C, N)                     # C=256, N=4096
    q = BN(Wq @ xf + bq)   -> [32, N]
    k = BN(Wk @ xf + bk)   -> [32, N]
    v = BN(Wv @ xf + bv)   -> [256, N]
    attn = softmax_j(q^T k) -> [N, N]
    out = v @ attn^T        -> [256, N]
    y = gamma * out + xf

Sharding: 8 cores = 4 batches x 2 query-row halves; no collectives.

Host-side algebra folds (all exact):
  - BN folded into weights/bias.
  - k bias drops entirely (adds a per-query constant to every logit row ->
    cancels in softmax); q bias kept (its logit term varies over keys).
  - v bias passes through softmax unchanged (rows sum to 1), so
    y = (gamma*vhat @ P)*recip + (x + gamma*bv); gamma folds into Wv and the
    residual bias rb = gamma*bv is added per-channel in the epilogue.
  - Each core's xbh has its own query-half columns FIRST (keys are
    order-invariant), so the query slice is a view of the key tensor and the
    bf16 x tile doubles as the residual.

Device (per core, all matmuls bf16 in / f32 psum):
  - Input DMAs in 3 column waves issued from three different queues
    (sync/gpsimd/scalar) so the ~0.7us per-issue cost is paid in parallel
    and the first projection matmul starts ~9.5us in.
  - q/k proj: 2 col-group replica matmuls -> [64, N*] (col-group pairs
    stream near-concurrently on the PE); q evicted via ACT Identity+bias,
    k via DVE copy.  Bands 64-127 are then filled by one SBUF->SBUF DMA
    per tensor, so S rounds can use 4 distinct PE row bands.
  - V^T[j, c] = x^T Wv^T computed directly in [j, c] layout.
  - Main loop over 4 i-blocks x 16 rounds of 2 key-chunks: S^T row-band
    matmul pair -> exp on ScalarE (no max subtraction; |S|<60 fits f32/bf16)
    -> P^T bf16 -> out psum accumulation + bf16 rowsum chains on DVE.
    Consecutive rounds use different PE row-band pairs (0/32 vs 64/96), so
    the adjacently emitted S pairs of rounds sr+2 and sr+3 stream 4-way
    concurrently.
  - Epilogue per i-block is split so no PE instruction ever waits on the
    rowsum->reciprocal chain: at the block's last round only the psum
    eviction happens; the rowsum matmuls + reciprocal + GpSimd
    partition_broadcast flush after the NEXT block's round-0 outs, and the
    final (out*recip + rb) + x + DMA after round-1 outs.
"""

import numpy as np
from contextlib import ExitStack

import ml_dtypes
import concourse.bass as bass
import concourse.bass_isa as bass_isa
import concourse.mybir as mybir
import concourse.tile as tile
from concourse import bacc
from concourse.bass_utils import run_bass_kernel_spmd

B, C, H, W = 4, 256, 64, 64
N = H * W            # 4096 tokens per image
CQ = C // 8          # 32 q/k channels
NQ = N // 2          # 2048 query tokens per core
EPS = 1e-5
P = 128
IB = 512             # i-block (psum bank of f32)
NJC = N // P         # 32 j-chunks
NSR = NJC // 2       # 16 rounds per i-block (2 j-chunks each)
NCORES = 8
NWARM = 46

f32 = mybir.dt.float32
bf16 = mybir.dt.bfloat16
FT = mybir.ActivationFunctionType
ALU = mybir.AluOpType
BF = ml_dtypes.bfloat16

_CACHE = {}


def _build():
    nc = bacc.Bacc("TRN2", target_bir_lowering=False, debug=False,
                   num_devices=NCORES)
    xbh = nc.dram_tensor("xbh", [C, N], bf16, kind="ExternalInput").ap()
    wqT = nc.dram_tensor("wqT", [C, CQ], bf16, kind="ExternalInput").ap()
    wkT = nc.dram_tensor("wkT", [C, CQ], bf16, kind="ExternalInput").ap()
    wvT = nc.dram_tensor("wvT", [C, C], bf16, kind="ExternalInput").ap()
    bq2 = nc.dram_tensor("bq2", [2 * CQ, 1], f32, kind="ExternalInput").ap()
    rbh = nc.dram_tensor("rbh", [P, 2], f32, kind="ExternalInput").ap()
    y = nc.dram_tensor("y", [C, NQ], bf16, kind="ExternalOutput").ap()

    with tile.TileContext(nc) as tc, ExitStack() as ctx:
        const = ctx.enter_context(tc.tile_pool(name="const", bufs=1))
        ones_col = const.tile([P, 1], bf16)
        nc.vector.memset(ones_col[:], 1.0)
        ones_row = const.tile([1, P], bf16)
        nc.vector.memset(ones_row[:], 1.0)

        wq_sb = const.tile([P, 2, CQ], bf16)
        wk_sb = const.tile([P, 2, CQ], bf16)
        wv_sb = const.tile([P, 2, C], bf16)
        bq_sb = const.tile([2 * CQ, 1], f32)
        rb_sb = const.tile([P, 2], f32)

        garb = const.tile([P, 64], bf16)
        nc.vector.memset(garb[:], 0.5)

        xpool = ctx.enter_context(tc.tile_pool(name="x", bufs=1))
        xb_sb = [xpool.tile([P, N], bf16, name=f"xbsb{cc}") for cc in range(2)]

        qkv = ctx.enter_context(tc.tile_pool(name="qkv", bufs=1))
        qrep = qkv.tile([P, NQ], bf16)   # 4 row-band replicas of q
        krep = qkv.tile([P, N], bf16)    # 4 row-band replicas of k
        vt_sb = qkv.tile([P, NJC, C], bf16)  # V^T as [j-in-chunk, jc, c]

        # ---- input DMA: 4 column waves, xb0 on sync / xb1 on gpsimd /
        # weights on scalar.  Each wave overlaps the next by ONE column:
        # the WAW forces wave w+1 to start only after wave w completes, so
        # the first wave drains at full DMA bandwidth instead of sharing
        # it with the whole input, and the q-proj can start ~2us earlier.
        XW = (0, 512, 1536, 2816, N)
        nc.scalar.dma_start(out=wq_sb[:],
                            in_=wqT.rearrange("(k p) m -> p k m", k=2))
        for w in range(4):
            hi = min(XW[w + 1] + 1, N)
            nc.sync.dma_start(out=xb_sb[0][:, XW[w]:hi],
                              in_=xbh[0:P, XW[w]:hi])
            nc.gpsimd.dma_start(out=xb_sb[1][:, XW[w]:hi],
                                in_=xbh[P:C, XW[w]:hi])
        nc.scalar.dma_start(out=bq_sb[:], in_=bq2[:])
        nc.scalar.dma_start(out=wk_sb[:],
                            in_=wkT.rearrange("(k p) m -> p k m", k=2))
        nc.scalar.dma_start(out=wv_sb[:],
                            in_=wvT.rearrange("(k p) m -> p k m", k=2))
        nc.scalar.dma_start(out=rb_sb[:], in_=rbh[:])

        # PE warmup during the input-DMA window: dependency-free tiny
        # matmuls keep the PE busy past the HAM activity window so the
        # projection phase starts at the full clock.
        with tc.tile_pool(name="warm_ps", bufs=1, space="PSUM") as wps:
            wtile = wps.tile([1, 64], f32, name="warm")
            for _ in range(NWARM):
                nc.tensor.matmul(wtile[:], lhsT=ones_col[:], rhs=garb[:],
                                 start=True, stop=True)

        # ---- projections, emitted in column-availability order so the
        # PE is never waiting on a later DMA wave while earlier-column
        # work exists ----
        def emit_q(nb, pps):
            # single col-group: replicas to the other 3 PE row bands come
            # from SBUF->SBUF DMA, not extra matmul streams
            ps = pps.tile([CQ, IB], f32, name="qps", tag="pps")
            for k in range(2):
                nc.tensor.matmul(
                    ps[:],
                    lhsT=wq_sb[:, k, :],
                    rhs=xb_sb[k][:, nb * IB:(nb + 1) * IB],
                    start=(k == 0), stop=(k == 1))
            nc.scalar.activation(
                out=qrep[0:CQ, nb * IB:(nb + 1) * IB], in_=ps[:],
                func=FT.Identity, bias=bq_sb[0:CQ, :])

        def emit_k(nb, pps):
            ps = pps.tile([CQ, IB], f32, name="kps", tag="pps")
            for k in range(2):
                nc.tensor.matmul(
                    ps[:],
                    lhsT=wk_sb[:, k, :],
                    rhs=xb_sb[k][:, nb * IB:(nb + 1) * IB],
                    start=(k == 0), stop=(k == 1))
            nc.vector.tensor_copy(
                out=krep[0:CQ, nb * IB:(nb + 1) * IB], in_=ps[:])

        def emit_v(jc, pps):
            # V^T = x^T Wv^T in [j, c] layout (gamma folded into Wv on host)
            ps = pps.tile([P, C], f32, name="vps", tag="pps")
            for k in range(2):
                nc.tensor.matmul(
                    ps[:],
                    lhsT=xb_sb[k][:, jc * P:(jc + 1) * P],
                    rhs=wv_sb[:, k, :],
                    start=(k == 0), stop=(k == 1))
            nc.vector.tensor_copy(out=vt_sb[:, jc, :], in_=ps[:])

        with tc.tile_pool(name="proj_ps", bufs=4, space="PSUM") as pps, \
                tc.tile_pool(name="pad_ps", bufs=1, space="PSUM") as pad_ps:
            padt = pad_ps.tile([1, 64], f32, name="padt")

            def pad(n):
                # dependency-free filler matmuls: keep the PE continuously
                # busy across DMA-wave seams so the pstate never drops back
                # to the slow ramp
                for _ in range(n):
                    nc.tensor.matmul(padt[:], lhsT=ones_col[:], rhs=garb[:],
                                     start=True, stop=True)

            for step in ("q0", "v0", "v1", "v2", "v3", "P8",
                         "q1", "q2", "v4", "v5", "v6", "v7", "v8", "v9",
                         "v10", "v11", "P8", "q3", "QR1", "QR2",
                         "k0", "k1", "k2", "v12", "v13", "v14", "v15",
                         "k3", "KRa1", "k4", "P8", "v16", "v17", "v18", "v19",
                         "v20", "v21", "k5", "KRa2", "k6", "k7", "KRb1",
                         "KRb2",
                         "v22", "v23", "v24", "v25", "v26", "v27", "v28",
                         "v29", "v30", "v31"):
                if step == "P8":
                    pad(8)
                elif step == "QR1":   # q band 0 -> 1
                    nc.gpsimd.dma_start(out=qrep[CQ:2 * CQ, :],
                                        in_=qrep[0:CQ, :])
                elif step == "QR2":   # q bands 0/1 -> 2/3
                    nc.gpsimd.dma_start(out=qrep[2 * CQ:P, :],
                                        in_=qrep[0:2 * CQ, :])
                elif step == "KRa1":  # k band 0 -> 1, first half
                    nc.sync.dma_start(out=krep[CQ:2 * CQ, 0:2048],
                                      in_=krep[0:CQ, 0:2048])
                elif step == "KRa2":  # k bands 0/1 -> 2/3, first half
                    nc.sync.dma_start(out=krep[2 * CQ:P, 0:2048],
                                      in_=krep[0:2 * CQ, 0:2048])
                elif step == "KRb1":  # k band 0 -> 1, second half
                    nc.sync.dma_start(out=krep[CQ:2 * CQ, 2048:N],
                                      in_=krep[0:CQ, 2048:N])
                elif step == "KRb2":  # k bands 0/1 -> 2/3, second half
                    nc.sync.dma_start(out=krep[2 * CQ:P, 2048:N],
                                      in_=krep[0:2 * CQ, 2048:N])
                elif step[0] == "q":
                    emit_q(int(step[1:]), pps)
                elif step[0] == "k":
                    emit_k(int(step[1:]), pps)
                else:
                    emit_v(int(step[1:]), pps)

        # ---- attention main loop ----
        sp_ps = ctx.enter_context(tc.tile_pool(name="sp_ps", bufs=3, space="PSUM"))
        out_ps = ctx.enter_context(tc.tile_pool(name="out_ps", bufs=1, space="PSUM"))
        pp_pool = ctx.enter_context(tc.tile_pool(name="ppp", bufs=4))
        acc_pool = ctx.enter_context(tc.tile_pool(name="accp", bufs=4))
        osb_pool = ctx.enter_context(tc.tile_pool(name="osbp", bufs=2))
        ysb_pool = ctx.enter_context(tc.tile_pool(name="ysbp", bufs=4))
        rec_pool = ctx.enter_context(tc.tile_pool(name="recp", bufs=2))
        bc_pool = ctx.enter_context(tc.tile_pool(name="bcp", bufs=2))

        NIB = NQ // IB
        NSRT = NIB * NSR   # total rounds

        def emit_s(sr):
            """S^T matmul pair for round sr: two concurrent 32-row-band MMs.

            Even rounds use PE row bands 0/32, odd rounds 64/96, so the two
            adjacently emitted pairs stream 4-way concurrently.
            """
            ib, lsr = divmod(sr, NSR)
            i0 = ib * IB
            base = (sr % 2) * 2 * CQ
            sp = sp_ps.tile([P, 2, IB], f32, name="sp")
            for g in range(2):
                jc = 2 * lsr + g
                bb = base + g * CQ
                nc.tensor.matmul(
                    sp[:, g, :],
                    lhsT=krep[bb:bb + CQ, jc * P:(jc + 1) * P],
                    rhs=qrep[bb:bb + CQ, i0:i0 + IB],
                    start=True, stop=True,
                    tile_position=(bb, 0))
            return sp

        def flush1(st):
            """Both rowsum chains + reciprocal + broadcast.  The psum epi
            tile is allocated HERE (after the next block's round-0 outs), so
            it never pinches the sp slot rotation across the boundary."""
            accA, accB = st["accA"], st["accB"]
            epi = sp_ps.tile([P, 2, IB], f32, name="sp")
            rs = epi[0:1, 0, :]
            for g in range(2):
                nc.tensor.matmul(rs, lhsT=ones_col[:], rhs=accA[:, g, :],
                                 start=(g == 0), stop=False)
            for g in range(2):
                nc.tensor.matmul(rs, lhsT=ones_col[:], rhs=accB[:, g, :],
                                 start=False, stop=(g == 1))
            recip = rec_pool.tile([1, IB], f32, name="recip")
            nc.vector.reciprocal_approx_fast(out=recip[:], in_=rs)
            recb = rec_pool.tile([1, IB], bf16, name="recb")
            nc.vector.tensor_copy(out=recb[:], in_=recip[:])
            if st["last"]:
                bc = epi[:, 1, :]
                nc.tensor.matmul(bc, lhsT=ones_row[:], rhs=recb[:],
                                 start=True, stop=True)
                bcs = bc_pool.tile([P, IB], bf16, name="bcs")
                nc.vector.tensor_copy(out=bcs[:], in_=bc)
            else:
                bcs = bc_pool.tile([P, IB], bf16, name="bcs")
                nc.gpsimd.partition_broadcast(out_ap=bcs[:], in_ap=recb[:],
                                              channels=P)
            st["bcs"] = bcs

        def flush2(st):
            """Normalize + residual + output DMA, all bf16 so DVE runs in
            2x mode and the boundary rounds don't stall the exp stream."""
            bcs, osb, i0 = st["bcs"], st["osb"], st["i0"]
            for cc in range(2):
                tmp = ysb_pool.tile([P, IB], bf16, name="tmp")
                nc.vector.tensor_mul(tmp[:], osb[:, cc, :], bcs[:])
                ysb = ysb_pool.tile([P, IB], bf16, name="ysb")
                nc.vector.scalar_tensor_tensor(
                    out=ysb[:], in0=tmp[:], scalar=rb_sb[:, cc:cc + 1],
                    in1=xb_sb[cc][:, i0:i0 + IB],
                    op0=ALU.add, op1=ALU.add)
                nc.sync.dma_start(out=y[cc * P:(cc + 1) * P, i0:i0 + IB],
                                  in_=ysb[:])

        # software pipeline: S for round sr+2 is emitted right after round
        # sr's out matmuls, so its LDWEIGHTS prefetch during them and the
        # PE never waits on the exp stream.
        sps = {0: emit_s(0), 1: emit_s(1)}
        outp = None
        accs = None
        pending = []
        for sr in range(NSRT):
            ib, lsr = divmod(sr, NSR)
            i0 = ib * IB
            if lsr == 0:
                outp = out_ps.tile([P, 2, IB], f32, name="outp")
                accs = [None, None]
            ch = lsr // (NSR // 2)   # rowsum chain A: rounds 0-7, B: 8-15
            if lsr % (NSR // 2) == 0:
                # chain start: exp writes the accumulator tile directly
                ppt = acc_pool.tile([P, 2, IB], bf16, name=f"acc{ch}")
                accs[ch] = ppt
                nc.scalar.activation(out=ppt[:], in_=sps.pop(sr)[:],
                                     func=FT.Exp)
            else:
                ppt = pp_pool.tile([P, 2, IB], bf16, name="pp")
                nc.scalar.activation(out=ppt[:], in_=sps.pop(sr)[:],
                                     func=FT.Exp)
                nc.vector.tensor_add(accs[ch][:], accs[ch][:], ppt[:])
            # S pairs for rounds sr+2 and sr+3 are emitted together (one
            # tile-mode run, LDWs pipeline) before this round's out matmuls;
            # with sp bufs=3 their psum WAR is already satisfied, so the exp
            # stream never waits on S production.
            if lsr % 2 == 0:
                for dd in (2, 3):
                    if sr + dd < NSRT:
                        sps[sr + dd] = emit_s(sr + dd)
            for g in range(2):
                jc = 2 * lsr + g
                for cc in range(2):
                    nc.tensor.matmul(
                        outp[:, cc, :],
                        lhsT=vt_sb[:, jc, cc * P:(cc + 1) * P],
                        rhs=ppt[:, g, :],
                        start=(lsr == 0 and g == 0),
                        stop=(lsr == NSR - 1 and g == 1))
            # previous block's deferred epilogue: the reduction after this
            # block's round-0 outs, the normalize+store after round-2 outs
            # (the recip->broadcast chain takes ~2us to produce bcs).
            if pending and ((lsr == 0 and len(pending) == 2) or lsr == 2):
                st, fn = pending.pop(0)
                fn(st)
            if lsr < NSR - 1:
                continue
            # ---- end of i-block: evict psum, defer the rest ----
            # evict psum so next i-block's matmuls proceed (one fused DVE
            # copy: cheaper than two, and ACT must stay on the exp stream)
            osb = osb_pool.tile([P, 2, IB], bf16, name="osb")
            nc.vector.tensor_copy(out=osb[:], in_=outp[:])
            st = {"accA": accs[0], "accB": accs[1], "osb": osb, "i0": i0,
                  "last": sr == NSRT - 1}
            pending.append((st, flush1))
            pending.append((st, flush2))
        for st, fn in pending:   # last i-block epilogue
            fn(st)

    nc.compile()
    return nc


def _get_nc():
    if "nc" not in _CACHE:
        _CACHE["nc"] = _build()
    return _CACHE["nc"]


def _fold_bn(w, b, g, beta, mean, var):
    s = g / np.sqrt(var + EPS)
    return w * s[:, None], b * s + beta - mean * s


def _in_maps(inputs):
    gx = np.asarray(inputs["x"], np.float32)
    gamma = float(np.asarray(inputs["gamma"]).reshape(-1)[0])
    wq, bq_ = _fold_bn(*[np.asarray(inputs[k], np.float32) for k in
                         ("q_w", "q_b", "q_g", "q_beta", "q_mean", "q_var")])
    wk, _bk = _fold_bn(*[np.asarray(inputs[k], np.float32) for k in
                         ("k_w", "k_b", "k_g", "k_beta", "k_mean", "k_var")])
    wv, bv_ = _fold_bn(*[np.asarray(inputs[k], np.float32) for k in
                         ("v_w", "v_b", "v_g", "v_beta", "v_mean", "v_var")])
    wqT = np.ascontiguousarray(wq.T.astype(BF))
    wkT = np.ascontiguousarray(wk.T.astype(BF))
    wvT = np.ascontiguousarray((gamma * wv).T.astype(BF))
    bq2 = np.ascontiguousarray(np.tile(bq_.reshape(CQ, 1), (2, 1)))
    rbh = np.ascontiguousarray((gamma * bv_).reshape(2, P).T)
    maps = []
    for core in range(NCORES):
        b, h = divmod(core, 2)
        xf = gx[b].reshape(C, N).astype(BF)
        if h == 1:  # own query-half columns first; key order is irrelevant
            xf = np.concatenate([xf[:, NQ:], xf[:, :NQ]], axis=1)
        maps.append({
            "xbh": np.ascontiguousarray(xf),
            "wqT": wqT, "wkT": wkT, "wvT": wvT,
            "bq2": bq2, "rbh": rbh,
        })
    return maps


def _gather(results):
    out = np.empty((B, C, N), np.float32)
    for core in range(NCORES):
        b, h = divmod(core, 2)
        out[b][:, h * NQ:(h + 1) * NQ] = np.asarray(
            results[core]["y"]).astype(np.float32)
    return out.reshape(B, C, H, W)


def _run(inputs, **kw):
    nc = _get_nc()
    res = run_bass_kernel_spmd(nc, _in_maps(inputs),
                               core_ids=list(range(NCORES)), **kw)
    return res


def kernel(**inputs) -> np.ndarray:
    return _gather(_run(inputs).results)
